# revision 7
# baseline (speedup 1.0000x reference)
"""Additive (Bahdanau) attention log-softmax weights on 8 TRN2 NeuronCores.

Math (per batch b, head 0):
    qp = Q @ Wq^T ; kp = K @ Wk^T          (Wc = [Wq | Wk], both [D, D])
    logit[q, k] = Wl . tanh(qp[q] + kp[k] + bc) + bl + where(mask[k]==0, -1e9, 1.0)
    out[q, :]   = log_softmax(logit[q, :])

Distribution: pure data parallel, core c <- (batch b = c//2, q-half c%2),
no collectives.  Sparse-attention trick: keys with mask==0 only need
out = -1e9 - LSE (error O(1) vs magnitude 1e9), so the device only computes
tanh over the ~136 valid keys (host compacts + pads to V).

Device layout per core (q = 128 local rows, V padded valid keys, e = D = 512):
  - PE: qp^T/kp^T projections ([e,q],[e,kc]) with d on partitions.
  - DVE: arg[e, (q,kc)] = kp^T[e,kc] + (qp^T+bc)[e,q] via per-q tensor_scalar
    (bf16, 4x mode).
  - ACT: tanh on [128, 32*V] tiles (big free dim amortizes overhead).
  - PE: Wl-reduce per q-pair into PSUM rows at partition bases {0,32,64,96}.
  - DVE copy PSUM->SBUF, DMA row-gather to a dense [64, 2*V] logits tile,
    then log-softmax (exp is safe without max subtraction: |logit| <= ~8).
"""

import numpy as np
import ml_dtypes
from contextlib import ExitStack

import concourse.bass as bass
import concourse.tile as tile
from concourse import bacc, mybir
from concourse.bass_utils import run_bass_kernel_spmd

F32 = mybir.dt.float32
BF16 = mybir.dt.bfloat16
AF = mybir.ActivationFunctionType

B, H, Lq, Lkv, D = 4, 1, 256, 256, 512
NCORES = 8
LQL = Lq // 2          # q rows per core
G = 32                 # q rows per tanh tile
NEG = -1.0e9

_nc_cache: dict[int, object] = {}


def _build(V: int):
    """Build + schedule the per-core Bass graph for padded-valid-count V."""
    W = 2 * V
    nc = bacc.Bacc(None, target_bir_lowering=False)

    p_qt = nc.declare_dram_parameter("qt", [D, LQL], BF16, isOutput=False)
    p_kt = nc.declare_dram_parameter("kt", [D, V], BF16, isOutput=False)
    p_wct = nc.declare_dram_parameter("wct", [2 * D, D], BF16, isOutput=False)
    p_bcp = nc.declare_dram_parameter("bcp", [128, 4], F32, isOutput=False)
    p_wlg = nc.declare_dram_parameter("wlg", [128, 256], BF16, isOutput=False)
    p_bv = nc.declare_dram_parameter("bv", [64, W], F32, isOutput=False)
    p_out = nc.declare_dram_parameter("out", [64, W + 2], F32, isOutput=True)

    with ExitStack() as ctx:
        tc = ctx.enter_context(tile.TileContext(nc))
        const = ctx.enter_context(tc.tile_pool(name="const", bufs=1))
        apool = ctx.enter_context(tc.tile_pool(name="apool", bufs=3))
        tpool = ctx.enter_context(tc.tile_pool(name="tpool", bufs=5))
        spool = ctx.enter_context(tc.tile_pool(name="spool", bufs=4))
        psum = ctx.enter_context(tc.tile_pool(name="psum", bufs=8, space="PSUM"))

        # ---- loads ----
        wct_t = []
        for i in range(8):
            t = const.tile([128, D], BF16, tag=f"wct{i}")
            nc.sync.dma_start(t[:], p_wct[i * 128:(i + 1) * 128, :])
            wct_t.append(t)
        qt_t = []
        for dc in range(4):
            t = const.tile([128, LQL], BF16, tag=f"qt{dc}")
            nc.sync.dma_start(t[:], p_qt[dc * 128:(dc + 1) * 128, :])
            qt_t.append(t)
        kt_t = []
        for dc in range(4):
            t = const.tile([128, V], BF16, tag=f"kt{dc}")
            nc.sync.dma_start(t[:], p_kt[dc * 128:(dc + 1) * 128, :])
            kt_t.append(t)
        bcp_t = const.tile([128, 4], F32, tag="bcp")
        nc.sync.dma_start(bcp_t[:], p_bcp[:])
        wlg_t = const.tile([128, 256], BF16, tag="wlg")
        nc.sync.dma_start(wlg_t[:], p_wlg[:])
        bv_t = const.tile([64, W], F32, tag="bv")
        nc.sync.dma_start(bv_t[:], p_bv[:])

        # ---- phase 1: projections (d on partitions -> [e, q] / [e, kc]) ----
        qpbc = const.tile([128, D], F32, tag="qpbc")   # col = ec*128 + q
        kpb = const.tile([128, 4 * V], BF16, tag="kpb")  # col = ec*V + kc
        for ec in range(4):
            ps = psum.tile([128, 128], F32, tag="ps")
            for dc in range(4):
                nc.tensor.matmul(
                    ps[:], wct_t[dc][:, ec * 128:(ec + 1) * 128], qt_t[dc][:],
                    start=(dc == 0), stop=(dc == 3))
            nc.vector.tensor_scalar_add(
                qpbc[:, ec * 128:(ec + 1) * 128], ps[:], bcp_t[:, ec:ec + 1])
        for ec in range(4):
            ps = psum.tile([128, V], F32, tag="ps")
            for dc in range(4):
                nc.tensor.matmul(
                    ps[:], wct_t[4 + dc][:, ec * 128:(ec + 1) * 128], kt_t[dc][:],
                    start=(dc == 0), stop=(dc == 3))
            nc.vector.tensor_copy(kpb[:, ec * V:(ec + 1) * V], ps[:])

        # ---- phase 2: tanh + Wl-reduce ----
        lg = const.tile([64, W], F32, tag="lg")  # row = q-pair, col = (q%2)*V + kc
        for qg in range(LQL // G):
            thts = []
            for ec in range(4):
                a = apool.tile([128, G * V], BF16, tag="arg")
                for j in range(G):
                    q = qg * G + j
                    nc.vector.tensor_scalar_add(
                        a[:, j * V:(j + 1) * V],
                        kpb[:, ec * V:(ec + 1) * V],
                        qpbc[:, ec * 128 + q: ec * 128 + q + 1])
                t = tpool.tile([128, G * V], BF16, tag="tht")
                nc.scalar.activation(t[:], a[:], AF.Tanh)
                thts.append(t)
            ptiles = [psum.tile([128, W], F32, tag="ps", name=f"pt{qg}_{i}") for i in range(8)]
            for p_loc in range(G // 2):
                tl, r = p_loc // 2, p_loc % 2
                dst = ptiles[tl][64 * r: 64 * r + 64, :]
                for ec in range(4):
                    nc.tensor.matmul(
                        dst, wlg_t[:, ec * 64:ec * 64 + 64],
                        thts[ec][:, (2 * p_loc) * V:(2 * p_loc + 2) * V],
                        start=(ec == 0), stop=(ec == 3),
                        # the sim's zero-region tracker ignores the partition
                        # base, so the two disjoint 64-row groups per bank
                        # falsely collide; per-partition accumulation state is
                        # independent on HW
                        skip_group_check=True)
            for tl in range(8):
                st = spool.tile([128, W], F32, tag="st")
                nc.vector.tensor_copy(st[:], ptiles[tl][:])
                base = qg * (G // 2) + tl * 2
                nc.sync.dma_start(lg[base: base + 2, :], st[0:128:64, :])

        # ---- softmax (no max subtraction: |logit| bounded by sum|Wl|+1 < 9) ----
        lgb = const.tile([64, W], F32, tag="lgb")
        nc.vector.tensor_add(lgb[:], lg[:], bv_t[:])
        ex = const.tile([64, W], F32, tag="ex")
        nc.scalar.activation(ex[:], lgb[:], AF.Exp)
        sm = const.tile([64, 2], F32, tag="sm")
        nc.vector.tensor_reduce(
            sm[:], ex[:].rearrange("p (two v) -> p two v", two=2),
            axis=mybir.AxisListType.X, op=mybir.AluOpType.add)
        lsm = const.tile([64, 2], F32, tag="lsm")
        nc.scalar.activation(lsm[:], sm[:], AF.Ln)
        outv = const.tile([64, W], F32, tag="outv")
        nc.vector.tensor_scalar_sub(outv[:, 0:V], lgb[:, 0:V], lsm[:, 0:1])
        nc.vector.tensor_scalar_sub(outv[:, V:W], lgb[:, V:W], lsm[:, 1:2])
        fill = const.tile([64, 2], F32, tag="fill")
        nc.vector.tensor_scalar(
            fill[:], lsm[:], -1.0, NEG,
            op0=mybir.AluOpType.mult, op1=mybir.AluOpType.add)
        nc.sync.dma_start(p_out[:, 0:W], outv[:])
        nc.sync.dma_start(p_out[:, W:W + 2], fill[:])

    nc.compile()
    return nc


def _prep(queries, keys, values, mask, Wc, bc, Wl, bl):
    """Host-side sharding: returns (V, in_maps, idx_valid, idx_masked)."""
    mask = np.asarray(mask)
    idx_v = [np.nonzero(mask[b])[0] for b in range(B)]
    idx_m = [np.nonzero(mask[b] == 0)[0] for b in range(B)]
    maxv = max(len(ix) for ix in idx_v)
    V = max(144, -(-maxv // 16) * 16)

    bf = ml_dtypes.bfloat16
    wct = np.ascontiguousarray(np.asarray(Wc, np.float32).T).astype(bf)  # [2D, D]
    bcp = np.ascontiguousarray(np.asarray(bc, np.float32).reshape(4, 128).T)
    wlg = np.zeros((128, 256), ml_dtypes.bfloat16)
    wlg[:, 0::64] = np.asarray(Wl, np.float32)[0].reshape(4, 128).T.astype(bf)
    blv = float(np.asarray(bl, np.float32)[0])

    q_np = np.asarray(queries, np.float32)
    k_np = np.asarray(keys, np.float32)
    in_maps = []
    for c in range(NCORES):
        b, qh = c // 2, c % 2
        qt = np.ascontiguousarray(
            q_np[b, 0, qh * LQL:(qh + 1) * LQL, :].T).astype(bf)  # [D, LQL]
        ktc = np.zeros((D, V), bf)
        ktc[:, :len(idx_v[b])] = k_np[b, 0, idx_v[b], :].T.astype(bf)
        bvrow = np.full(V, NEG, np.float32)
        bvrow[:len(idx_v[b])] = 1.0 + blv
        bv = np.tile(np.concatenate([bvrow, bvrow]), (64, 1))
        in_maps.append({
            "qt": qt, "kt": np.ascontiguousarray(ktc), "wct": wct,
            "bcp": bcp, "wlg": wlg, "bv": np.ascontiguousarray(bv),
        })
    return V, in_maps, idx_v, idx_m


def kernel(queries, keys, values, mask, Wc, bc, Wl, bl):
    V, in_maps, idx_v, idx_m = _prep(queries, keys, values, mask, Wc, bc, Wl, bl)
    if V not in _nc_cache:
        _nc_cache[V] = _build(V)
    nc = _nc_cache[V]
    res = run_bass_kernel_spmd(nc, in_maps, core_ids=list(range(NCORES))).results

    W = 2 * V
    full = np.empty((B, Lq, Lkv), np.float32)
    for c in range(NCORES):
        b, qh = c // 2, c % 2
        o = np.asarray(res[c]["out"], np.float32)       # [64, W+2]
        vals = o[:, :W].reshape(64, 2, V)               # [pair, q%2, kc]
        fl = o[:, W:W + 2]                              # [pair, q%2]
        nv = len(idx_v[b])
        blk = full[b, qh * LQL:(qh + 1) * LQL]          # [128, Lkv]
        blk[:, idx_v[b]] = vals[:, :, :nv].reshape(LQL, nv)
        blk[:, idx_m[b]] = fl.reshape(LQL, 1)
    return full
